# revision 5
# baseline (speedup 1.0000x reference)
"""Trainium2 Bass kernel for nn_CognitiveWorkspaceTransformer.

Math (reference semantics):
    X   = S + concat(w_spoke, w_hub_priv, w_hub_shared, tag)   # full 1088 cover
    out = X @ W_read.T          # (B,T,1024)
    k   = latent @ Wk.T         # cache is fully overwritten by latent
    v   = latent @ Wv.T

Sharding: data-parallel over batch B=8, one batch element per NeuronCore.
All tensors are laid out feature-major on the host (pure layout prep plus a
bf16 downcast, no arithmetic) so the contraction dim lands on SBUF
partitions directly and the PE needs no on-chip transposes.

bf16 everywhere (tolerance is 2e-2; bf16 lands ~5e-3): ~47MB/core HBM
traffic (~131us roofline) vs ~360k PE cycles (~150us @ 2.4GHz) -> the PE
array is the bottleneck; everything else is scheduled to keep it fed:
  - warm-up junk matmuls at t=0 so the HAM clock gate reaches 2.4GHz
    before real work lands (otherwise early matmuls run at 1.2GHz)
  - slab 0 runs all k/v matmuls first (they only need the tiny Wk/Wv and
    latent loads), covering the W_read/S/wc load ramp with real work
  - j-outer/h-inner so each 128x128 stationary X^T chunk is loaded once
  - 2-bank PSUM tiles [128,1024] so each out/k/v tile needs ONE wide
    PSUM->SBUF cast-copy (out,v on DVE; k on ACT)
  - paired [256,1024] output stores halve DMA issue count
"""

import numpy as np
import ml_dtypes

import concourse.bacc as bacc
import concourse.mybir as mybir
import concourse.tile as tile
from concourse.bass_utils import run_bass_kernel_spmd

B, T, D_STATE, D_MODEL, D_LATENT = 8, 4096, 1088, 1024, 128
N_CORES = 8
P = 128
F32 = mybir.dt.float32
BF16 = mybir.dt.bfloat16

# feature chunks of the contraction dim (1088 = 8*128 + 64)
R_CHUNKS = [(j * 128, min(128, D_STATE - j * 128)) for j in range((D_STATE + 127) // 128)]
NJ = len(R_CHUNKS)

_NC_CACHE = {}


def build_nc(mm_dt=BF16, out_dt=BF16, t_chunk=1024, in_bufs=3, wc_bufs=2, out_bufs=2,
             kv0_bufs=4, in_split=2, warmup_mms=12):
    """Build + compile the per-core Bass program (identical on all cores)."""
    slabs = [t_chunk] * (T // t_chunk)
    assert sum(slabs) == T

    nc = bacc.Bacc("TRN2", target_bir_lowering=False, debug=False, num_devices=N_CORES)

    # feature-major inputs: sT/wcT [1088, T], latT [128, T]
    st_d = nc.dram_tensor("st", [D_STATE, T], mm_dt, kind="ExternalInput").ap()
    wct_d = nc.dram_tensor("wct", [D_STATE, T], mm_dt, kind="ExternalInput").ap()
    latt_d = nc.dram_tensor("latt", [D_LATENT, T], mm_dt, kind="ExternalInput").ap()
    wrt_d = nc.dram_tensor("wrt", [D_STATE, D_MODEL], mm_dt, kind="ExternalInput").ap()
    wkt_d = nc.dram_tensor("wkt", [D_LATENT, D_MODEL], mm_dt, kind="ExternalInput").ap()
    wvt_d = nc.dram_tensor("wvt", [D_LATENT, D_MODEL], mm_dt, kind="ExternalInput").ap()
    out_d = nc.dram_tensor("out", [T, D_MODEL], out_dt, kind="ExternalOutput").ap()
    k_d = nc.dram_tensor("k", [T, D_MODEL], out_dt, kind="ExternalOutput").ap()
    v_d = nc.dram_tensor("v", [T, D_MODEL], out_dt, kind="ExternalOutput").ap()

    with tile.TileContext(nc) as tc:
        with (
            tc.tile_pool(name="weights", bufs=1) as wpool,
            tc.tile_pool(name="ins", bufs=in_bufs) as inpool,
            tc.tile_pool(name="wcp", bufs=wc_bufs) as wcpool,
            tc.tile_pool(name="outs", bufs=out_bufs) as outpool,
            tc.tile_pool(name="kv0", bufs=kv0_bufs) as kv0pool,
            tc.tile_pool(name="psum_out", bufs=2, space="PSUM") as pout_pool,
            tc.tile_pool(name="psum_kv", bufs=1, space="PSUM") as pkv_pool,
        ):
            # tiny weights + whole latT first on scalar queue: k/v matmuls can
            # start ~2us in, while W_read/S/wc are still streaming
            wk_t = wpool.tile([D_LATENT, D_MODEL], mm_dt, tag="wk")
            nc.scalar.dma_start(wk_t[:], wkt_d[:])
            wv_t = wpool.tile([D_LATENT, D_MODEL], mm_dt, tag="wv")
            nc.scalar.dma_start(wv_t[:], wvt_d[:])
            lt = wpool.tile([D_LATENT, T], mm_dt, tag="lt")
            nc.scalar.dma_start(lt[:], latt_d[:])
            ltr = lt[:]

            # W_read chunks on sync queue
            wr_tiles = []
            for j, (r0, rw) in enumerate(R_CHUNKS):
                wt = wpool.tile([rw, D_MODEL], mm_dt, tag=f"wr{j}")
                nc.sync.dma_start(wt[:], wrt_d[r0 : r0 + rw, :])
                wr_tiles.append(wt)

            # HAM warm-up: junk matmuls on a zeroed scratch tile keep the PE
            # busy through the clock-gate window while the first loads land
            if warmup_mms:
                scratch = wpool.tile([P, 512], mm_dt, tag="scratch")
                nc.vector.memset(scratch[:], 0.0)
                pj = pout_pool.tile([P, D_MODEL], F32, tag="pout", name="pjunk")
                for w in range(warmup_mms):
                    nc.tensor.matmul(pj[:, 0:512], scratch[:, 0:P],
                                     scratch[:, 0:512], start=True, stop=True)
                # keep the junk matmuls live
                keep = wpool.tile([1, 8], F32, tag="keep")
                nc.vector.tensor_copy(keep[:], pj[0:1, 0:8])

            def emit_kv(ts_abs, k_sb, v_sb):
                pk = pkv_pool.tile([P, D_MODEL], F32, tag="pk", name="pk")
                pv = pkv_pool.tile([P, D_MODEL], F32, tag="pv", name="pv")
                for h in range(2):
                    n0 = h * 512
                    nc.tensor.matmul(
                        pk[:, n0 : n0 + 512],
                        ltr[:, ts_abs : ts_abs + P], wk_t[:, n0 : n0 + 512],
                        start=True, stop=True)
                    nc.tensor.matmul(
                        pv[:, n0 : n0 + 512],
                        ltr[:, ts_abs : ts_abs + P], wv_t[:, n0 : n0 + 512],
                        start=True, stop=True)
                nc.scalar.copy(k_sb, pk[:])
                nc.vector.tensor_copy(v_sb, pv[:])

            def emit_out(xr, ts0, out_sb):
                po = pout_pool.tile([P, D_MODEL], F32, tag="pout", name="po")
                for j, (r0, rw) in enumerate(R_CHUNKS):
                    for h in range(2):
                        nc.tensor.matmul(
                            po[:, h * 512 : h * 512 + 512],
                            xr[0:rw, j, ts0 : ts0 + P],
                            wr_tiles[j][0:rw, h * 512 : h * 512 + 512],
                            start=(j == 0),
                            stop=(j == NJ - 1),
                        )
                nc.vector.tensor_copy(out_sb, po[:])

            def store_rows(row0, dsts_tiles, g_pair):
                # dsts_tiles: list of (dram_ap, pair_tile); alternate queues
                eng = [nc.scalar, nc.sync] if g_pair % 2 == 0 else [nc.sync, nc.scalar]
                for i, (dram, tl) in enumerate(dsts_tiles):
                    dst = dram[row0 : row0 + 2 * P, :].rearrange(
                        "(g p) d -> p g d", p=P)
                    eng[i % 2].dma_start(dst, tl[:])

            t_cursor = 0
            for it, sz in enumerate(slabs):
                t0 = t_cursor
                t_cursor += sz
                ng = sz // P
                # X^T tile: [128 (r within chunk), 9 chunks, t_chunk]
                xt = inpool.tile([P, NJ, sz], mm_dt, tag="x")
                wc = wcpool.tile([P, NJ, sz], mm_dt, tag="wc")
                tsl = sz // in_split
                for u in range(in_split):
                    u0 = u * tsl
                    nc.sync.dma_start(
                        xt[:, 0:8, u0 : u0 + tsl],
                        st_d[0:1024, t0 + u0 : t0 + u0 + tsl].rearrange(
                            "(j p) t -> p j t", p=P),
                    )
                    nc.sync.dma_start(
                        xt[0:64, 8, u0 : u0 + tsl],
                        st_d[1024:1088, t0 + u0 : t0 + u0 + tsl])
                    nc.scalar.dma_start(
                        wc[:, 0:8, u0 : u0 + tsl],
                        wct_d[0:1024, t0 + u0 : t0 + u0 + tsl].rearrange(
                            "(j p) t -> p j t", p=P),
                    )
                    nc.scalar.dma_start(
                        wc[0:64, 8, u0 : u0 + tsl],
                        wct_d[1024:1088, t0 + u0 : t0 + u0 + tsl])
                xr = xt[:]

                if it == 0:
                    # Phase A: k/v for the whole slab (only needs wk/wv/lat),
                    # stores issued per pair as they complete. Dedicated
                    # deeper ring (kv0) so nothing waits on out-path tiles.
                    kv_tiles = {}
                    for g in range(ng):
                        if g % 2 == 0:
                            kv_tiles[g] = (
                                kv0pool.tile([P, 2, D_MODEL], out_dt, tag="k0",
                                             name="k0_pr"),
                                kv0pool.tile([P, 2, D_MODEL], out_dt, tag="v0",
                                             name="v0_pr"),
                            )
                        kp, vp = kv_tiles[g - g % 2]
                        emit_kv(t0 + g * P, kp[:, g % 2, :], vp[:, g % 2, :])
                        if g % 2 == 1:
                            store_rows(t0 + (g - 1) * P,
                                       [(k_d, kp), (v_d, vp)], g // 2)
                    # Phase B: adds then out-GEMMs
                    for g in range(ng):
                        sl = slice(g * P, (g + 1) * P)
                        nc.vector.tensor_add(xr[:, :, sl], xt[:, :, sl],
                                             wc[:, :, sl])
                    opair = None
                    for g in range(ng):
                        if g % 2 == 0:
                            opair = outpool.tile([P, 2, D_MODEL], out_dt,
                                                 tag="out", name="out_pr")
                        emit_out(xr, g * P, opair[:, g % 2, :])
                        if g % 2 == 1:
                            store_rows(t0 + (g - 1) * P, [(out_d, opair)], g // 2)
                else:
                    for g in range(ng):
                        sl = slice(g * P, (g + 1) * P)
                        nc.vector.tensor_add(xr[:, :, sl], xt[:, :, sl],
                                             wc[:, :, sl])
                    tiles = None
                    for g in range(ng):
                        if g % 2 == 0:
                            tiles = (
                                outpool.tile([P, 2, D_MODEL], out_dt, tag="out",
                                             name="out_pr"),
                                outpool.tile([P, 2, D_MODEL], out_dt, tag="k",
                                             name="k_pr"),
                                outpool.tile([P, 2, D_MODEL], out_dt, tag="v",
                                             name="v_pr"),
                            )
                        op, kp, vp = tiles
                        emit_out(xr, g * P, op[:, g % 2, :])
                        emit_kv(t0 + g * P, kp[:, g % 2, :], vp[:, g % 2, :])
                        if g % 2 == 1:
                            store_rows(t0 + (g - 1) * P,
                                       [(out_d, op), (k_d, kp), (v_d, vp)],
                                       g // 2)

    nc.compile()
    return nc


def _get_nc(**kw):
    key = tuple(sorted(kw.items()))
    if key not in _NC_CACHE:
        _NC_CACHE[key] = build_nc(**kw)
    return _NC_CACHE[key]


def make_in_maps(S, w_spoke, w_hub_priv, w_hub_shared, tag, W_read, cache, latent,
                 Wk, Wv):
    # host-side layout prep only (shard over batch, feature-major transposes,
    # bf16 downcast)
    bf = ml_dtypes.bfloat16
    wcat = np.concatenate(
        [np.asarray(w_spoke, np.float32), np.asarray(w_hub_priv, np.float32),
         np.asarray(w_hub_shared, np.float32), np.asarray(tag, np.float32)],
        axis=-1,
    )
    sT = np.ascontiguousarray(np.asarray(S, np.float32).transpose(0, 2, 1)).astype(bf)
    wcT = np.ascontiguousarray(wcat.transpose(0, 2, 1)).astype(bf)
    latT = np.ascontiguousarray(
        np.asarray(latent, np.float32).transpose(0, 2, 1)).astype(bf)
    wrt = np.ascontiguousarray(np.asarray(W_read, np.float32).T).astype(bf)
    wkt = np.ascontiguousarray(np.asarray(Wk, np.float32).T).astype(bf)
    wvt = np.ascontiguousarray(np.asarray(Wv, np.float32).T).astype(bf)
    return [
        {"st": sT[i], "wct": wcT[i], "latt": latT[i],
         "wrt": wrt, "wkt": wkt, "wvt": wvt}
        for i in range(N_CORES)
    ]


def kernel(S, w_spoke, w_hub_priv, w_hub_shared, tag, W_read, cache, latent, Wk, Wv,
           **build_kw):
    in_maps = make_in_maps(S, w_spoke, w_hub_priv, w_hub_shared, tag, W_read, cache,
                           latent, Wk, Wv)
    nc = _get_nc(**build_kw)
    res = run_bass_kernel_spmd(nc, in_maps, list(range(N_CORES)))
    out = np.stack([res.results[i]["out"].astype(np.float32) for i in range(N_CORES)])
    k = np.stack([res.results[i]["k"].astype(np.float32) for i in range(N_CORES)])
    v = np.stack([res.results[i]["v"].astype(np.float32) for i in range(N_CORES)])
    return (out, k, v)
